# revision 27
# baseline (speedup 1.0000x reference)
"""Multi-head self-attention block (B=2, N=2048, C=1024, H=16, D=64) + output
projection, sharded over 8 Trainium2 NeuronCores.

Sharding: core c handles batch b=c//4 and heads 4*(c%4)..4*(c%4)+3 (data +
head parallel).  The output projection is row-sharded over the input-channel
dim (each core multiplies its 256 attention channels into a full [N, 1024]
partial product); the 4 partials per batch are summed on the host (the
"all-reduce") and the bias is added there.

Device kernel layout (per core, fp32 throughout):
  - q, k are fed pre-transposed per head-pair: [128, N] tiles whose partition
    dim stacks the two heads' 64 attention dims.
  - scores_T[k_row, n] for a 128-row key chunk come from one K=64 matmul per
    head (the two heads run in disjoint PE row groups and overlap).
  - exp() on ScalarE (PSUM -> SBUF), no max-subtraction (|scores| <~ 50, safe
    in fp32).
  - AV: lhsT is v augmented with a ones column, so PSUM accumulates x^T
    unnormalized (rows 0-63 / 64-127) and the softmax denominator (row 64 for
    even heads, row 32 for odd heads) in the same accumulation group.
  - normalization: fast reciprocal of the denominator row, broadcast across
    partitions with a K=1 matmul, fused into the PSUM->SBUF evacuation.
  - projection: x^T chunks are the matmul lhsT directly; [N,256]@[256,1024]
    partial product is written out unreduced.
"""

import os
from contextlib import ExitStack

import ml_dtypes
import numpy as np

import concourse.bass as bass
import concourse.tile as tile
from concourse import bacc, mybir
from concourse._compat import with_exitstack
from concourse import bass_utils

F32 = mybir.dt.float32

B, N, C, H, D = 2, 2048, 1024, 16, 64
NCORES = 8
HPC = 4  # heads per core
NPAIR = HPC // 2


def _mm_dtypes():
    """PE dtypes for the three matmul groups.

    qk/proj: "f32" (exact, 4 cyc/col) or "f32r" (reduced precision, 1
    cyc/col).  av: additionally "bf16" — bf16 AV matmuls keep the PE's HAM
    clock gate warm (fp32r streaming does not count as PE activity), which
    doubles the effective PE clock for the whole kernel."""
    qk = os.environ.get("ATTN_KERNEL_QK_DT", "f32r")
    av = os.environ.get("ATTN_KERNEL_AV_DT", "bf16")
    pj = os.environ.get("ATTN_KERNEL_PJ_DT", "f16")
    m = {
        "f32": F32,
        "f32r": mybir.dt.float32r,
        "bf16": mybir.dt.bfloat16,
        "f16": mybir.dt.float16,
    }
    return m[qk], m[av], m[pj]


def _bcast_row(row_ap, nparts):
    """DRAM AP view replicating a 1D row across `nparts` partitions."""
    return bass.AP(
        tensor=row_ap.tensor,
        offset=row_ap.offset,
        ap=[[0, nparts], *row_ap.ap],
    )


@with_exitstack
def attention_body(ctx: ExitStack, tc: tile.TileContext, out, qt, kt, vp, wt):
    """Emit the per-core attention+projection program.

    APs (all fp32):
      out  [N, OW]          partial projection output
      qt   [NPAIR, 128, N]  q transposed, head pair stacked on partitions
      kt   [NPAIR, 128, N]  k transposed, same packing
      vp   [2*NPAIR, 128, NJ, 128]  v chunks as AV lhsT: for even heads v in
           cols 0:64 and ones in col 64; for odd heads v in cols 64:128 and
           ones in col 32 (so x^T lands on the partitions matching qt packing)
      wt   [NPAIR, 128, OW] proj_w slice, transposed to [channel, out]
    """
    nc = tc.nc
    P = 128
    pilot = int(os.environ.get("ATTN_KERNEL_PILOT", "0"))
    npair, _, n = qt.shape
    NJ = n // P          # key chunks
    HW = n // 2          # query half processed per inner loop
    NT = max(1, HW // 512)
    MS = HW // NT        # matmul free-dim chunk (<=512)
    OW = wt.shape[2]
    OT = max(1, OW // 512)
    OS = OW // OT

    sing = ctx.enter_context(tc.tile_pool(name="sing", bufs=1))
    probs_pool = ctx.enter_context(tc.tile_pool(name="probs", bufs=4))
    work = ctx.enter_context(tc.tile_pool(name="work", bufs=2))
    ost = ctx.enter_context(tc.tile_pool(name="ost", bufs=3))
    psum = ctx.enter_context(tc.tile_pool(name="psum", bufs=2, space="PSUM"))
    dram = ctx.enter_context(tc.tile_pool(name="dram", bufs=2, space="DRAM"))

    # HAM warm-up: dense plain-fp32 matmuls on a constant tile run during the
    # input DMA window (no data dependency) and lift the PE clock gate to
    # 2.4 GHz before the real f32r/bf16 stream begins
    nwarm = int(os.environ.get("ATTN_KERNEL_WARMUP", "6"))
    if nwarm:
        wtile = sing.tile([P, 512], F32, tag="warm", name="warm")
        nc.vector.memset(wtile, 1.0)
        pw = psum.tile([P, 512], F32, tag="ps", name="warmps")
        for w in range(nwarm):
            nc.tensor.matmul(
                pw, lhsT=wtile[:, 0:128], rhs=wtile, start=True, stop=True
            )

    qts, kts, wts, xts, vps = [], [], [], [], []
    for p in range(npair):
        t = sing.tile([P, n], qt.dtype, tag=f"qt{p}", name=f"qts{p}")
        nc.sync.dma_start(t, qt[p])
        qts.append(t)
        t = sing.tile([P, n], kt.dtype, tag=f"kt{p}", name=f"kts{p}")
        nc.sync.dma_start(t, kt[p])
        kts.append(t)
        t = sing.tile([P, OW], wt.dtype, tag=f"wt{p}", name=f"wts{p}")
        nc.sync.dma_start(t, wt[p])
        wts.append(t)
        xts.append(sing.tile([P, n], wt.dtype, tag=f"xt{p}", name=f"xts{p}"))
    for h in range(2 * npair):
        t = sing.tile([P, NJ, P], vp.dtype, tag=f"vp{h}", name=f"vps{h}")
        nc.sync.dma_start(t, vp[h])
        vps.append(t)

    for Hi in range(2):
        h0 = Hi * HW
        for p in range(npair):
            po = [
                psum.tile([P, HW], F32, tag="po", name=f"po{Hi}{p}{a}")
                for a in range(2)
            ]
            def emit_qk(j):
                # QK for both heads, emission interleaved by row group so the
                # PE can overlap the K=64 matmuls of disjoint row halves
                pss = [
                    psum.tile([P, HW], F32, tag="ps", name=f"ps{Hi}{p}{j}{a}")
                    for a in range(2)
                ]
                for t in range(NT):
                    for a in range(2):
                        rows = slice(a * 64, a * 64 + 64)
                        nc.tensor.matmul(
                            pss[a][:, t * MS : (t + 1) * MS],
                            lhsT=kts[p][rows, j * P : (j + 1) * P],
                            rhs=qts[p][rows, h0 + t * MS : h0 + (t + 1) * MS],
                            start=True,
                            stop=True,
                        )
                return pss

            # software pipeline: emit QK for chunk j+1 before AV of chunk j so
            # the PE always has ready work behind the exp-gated AV matmuls
            pss = emit_qk(0)
            for j in range(NJ):
                pbs = []
                for a in range(2):
                    pb = probs_pool.tile(
                        [P, HW], vp.dtype, tag="pb", name=f"pb{Hi}{p}{j}{a}"
                    )
                    nc.scalar.activation(pb, pss[a], mybir.ActivationFunctionType.Exp)
                    pbs.append(pb)
                if j + 1 < NJ:
                    pss = emit_qk(j + 1)
                for a in range(2):
                    for t in range(NT):
                        nc.tensor.matmul(
                            po[a][:, t * MS : (t + 1) * MS],
                            lhsT=vps[2 * p + a][:, j, :],
                            rhs=pbs[a][:, t * MS : (t + 1) * MS],
                            start=(j == 0),
                            stop=(j == NJ - 1),
                        )
            # Evacuate PSUM immediately (denoms on ScalarE, x^T on VectorE) so
            # the po accumulators free up for the next head pair; the
            # broadcast + reciprocal + normalize then run asynchronously.
            dn = work.tile([P, HW], F32, tag="rc", name=f"rc{Hi}{p}")
            nc.vector.tensor_copy(dn[64:65, :], po[0][64:65, :])
            nc.vector.tensor_copy(dn[32:33, :], po[1][32:33, :])
            xu = work.tile([P, HW], F32, tag="xu", name=f"xu{Hi}{p}")
            nc.vector.tensor_copy(xu[0:64, :], po[0][0:64, :])
            nc.vector.tensor_copy(xu[64:128, :], po[1][64:128, :])
            dsc = dram.tile([2, HW], F32, tag="dsc", name=f"dsc{Hi}{p}")
            nc.sync.dma_start(dsc[0:1, :], dn[64:65, :])
            nc.sync.dma_start(dsc[1:2, :], dn[32:33, :])
            rbd = work.tile([P, HW], F32, tag="rbd", name=f"rbd{Hi}{p}")
            nc.sync.dma_start(rbd[0:64, :], _bcast_row(dsc[0], 64))
            nc.sync.dma_start(rbd[64:128, :], _bcast_row(dsc[1], 64))
            rb = work.tile([P, HW], F32, tag="rb", name=f"rb{Hi}{p}")
            rscr = work.tile([P, HW], F32, tag="rscr", name=f"rscr{Hi}{p}")
            nc.vector.reciprocal_approx_accurate(rb, rbd, rscr)
            nc.vector.tensor_mul(xts[p][:, h0 : h0 + HW], xu, rb)
    # projection: emitted after all attention so attention work is always
    # available behind it in the PE queue
    for i in range(n // P):
        pp = psum.tile([P, OW], F32, tag="ps", name=f"pp{i}")
        for cc in range(npair):
            for t in range(OT):
                nc.tensor.matmul(
                    pp[:, t * OS : (t + 1) * OS],
                    lhsT=xts[cc][:, i * P : (i + 1) * P],
                    rhs=wts[cc][:, t * OS : (t + 1) * OS],
                    start=(cc == 0),
                    stop=(cc == npair - 1),
                )
        ot = ost.tile([P, OW], F32, tag="ot", name=f"ot{i}")
        if i % 2 == 0:
            nc.vector.tensor_copy(ot, pp)
        else:
            nc.scalar.copy(ot, pp)
        nc.sync.dma_start(out[i * P : (i + 1) * P, :], ot)



def build_module(n=N, ow=C, npair=NPAIR):
    qkd, avd, pjd = _mm_dtypes()
    nc = bacc.Bacc("TRN2", target_bir_lowering=False, debug=False, num_devices=NCORES)
    nj = n // 128
    qt = nc.dram_tensor("qt", [npair, 128, n], qkd, kind="ExternalInput")
    kt = nc.dram_tensor("kt", [npair, 128, n], qkd, kind="ExternalInput")
    vp = nc.dram_tensor("vp", [2 * npair, 128, nj, 128], avd, kind="ExternalInput")
    wt = nc.dram_tensor("wt", [npair, 128, ow], pjd, kind="ExternalInput")
    out = nc.dram_tensor("out", [n, ow], F32, kind="ExternalOutput")
    with tile.TileContext(nc) as tc:
        attention_body(tc, out.ap(), qt.ap(), kt.ap(), vp.ap(), wt.ap())
    nc.compile()
    return nc


def shard_inputs(q, k, v, proj_w):
    """Build the 8 per-core input maps from the full tensors."""
    q = np.asarray(q, dtype=np.float32)
    k = np.asarray(k, dtype=np.float32)
    v = np.asarray(v, dtype=np.float32)
    proj_w = np.asarray(proj_w, dtype=np.float32)
    b_, n_, c_ = q.shape
    h_ = k.shape[1]
    d_ = c_ // h_
    nj = n_ // 128
    # [B, H, D, N]
    _np_dt = {"f32": np.float32, "f32r": np.float32, "bf16": ml_dtypes.bfloat16,
              "f16": np.float16}
    qk_np = _np_dt[os.environ.get("ATTN_KERNEL_QK_DT", "f32r")]
    qh = np.ascontiguousarray(
        q.reshape(b_, n_, h_, d_).transpose(0, 2, 3, 1).astype(qk_np)
    )
    kh = np.ascontiguousarray(k.transpose(0, 1, 3, 2).astype(qk_np))
    in_maps = []
    for c in range(NCORES):
        b = c // 4
        hh0 = HPC * (c % 4)
        qt = np.ascontiguousarray(qh[b, hh0 : hh0 + HPC].reshape(NPAIR, 128, n_))
        kt = np.ascontiguousarray(kh[b, hh0 : hh0 + HPC].reshape(NPAIR, 128, n_))
        avd = os.environ.get("ATTN_KERNEL_AV_DT", "bf16")
        vp_np = ml_dtypes.bfloat16 if avd == "bf16" else np.float32
        vp = np.zeros((HPC, 128, nj, 128), vp_np)
        for hh in range(HPC):
            vv = v[b, hh0 + hh].reshape(nj, 128, d_).transpose(1, 0, 2)
            if hh % 2 == 0:
                vp[hh][:, :, 0:64] = vv
                vp[hh][:, :, 64] = 1.0
            else:
                vp[hh][:, :, 64:128] = vv
                vp[hh][:, :, 32] = 1.0
        ch0 = hh0 * d_
        pj_np = _np_dt[os.environ.get("ATTN_KERNEL_PJ_DT", "f16")]
        wt = np.ascontiguousarray(
            proj_w[:, ch0 : ch0 + HPC * d_].T.reshape(NPAIR, 128, c_).astype(pj_np)
        )
        in_maps.append({"qt": qt, "kt": kt, "vp": vp, "wt": wt})
    return in_maps


def reduce_outputs(results, proj_b):
    """Sum the per-core partial projections per batch and add the bias."""
    outs = [np.asarray(r["out"], dtype=np.float32) for r in results]
    full = np.stack(
        [outs[0] + outs[1] + outs[2] + outs[3], outs[4] + outs[5] + outs[6] + outs[7]]
    )
    return (full + np.asarray(proj_b, dtype=np.float32)[None, None, :]).astype(
        np.float32
    )


_NC_CACHE = {}


def _get_module():
    if "nc" not in _NC_CACHE:
        _NC_CACHE["nc"] = build_module()
    return _NC_CACHE["nc"]


def kernel(q, k, v, proj_w, proj_b):
    nc = _get_module()
    in_maps = shard_inputs(q, k, v, proj_w)
    trace = bool(int(os.environ.get("ATTN_KERNEL_TRACE", "0")))
    kwargs = {}
    tmpdir = os.environ.get("ATTN_KERNEL_TMPDIR")
    if trace and tmpdir:
        os.makedirs(tmpdir, exist_ok=True)
        kwargs["tmpdir"] = tmpdir
    res = bass_utils.run_bass_kernel_spmd(
        nc, in_maps, core_ids=list(range(NCORES)), trace=trace, **kwargs
    )
    if trace:
        _NC_CACHE["last_results"] = res
    return reduce_outputs(res.results, proj_b)


# revision 28
# speedup vs baseline: 1.4190x; 1.4190x over previous
"""Multi-head self-attention block (B=2, N=2048, C=1024, H=16, D=64) + output
projection, sharded over 8 Trainium2 NeuronCores.

Sharding: core c handles batch b=c//4 and heads 4*(c%4)..4*(c%4)+3 (data +
head parallel).  The output projection is row-sharded over the input-channel
dim (each core multiplies its 256 attention channels into a full [N, 1024]
partial product); the 4 partials per batch are summed on the host (the
"all-reduce") and the bias is added there.

Device kernel layout (per core, fp32 throughout):
  - q, k are fed pre-transposed per head-pair: [128, N] tiles whose partition
    dim stacks the two heads' 64 attention dims.
  - scores_T[k_row, n] for a 128-row key chunk come from one K=64 matmul per
    head (the two heads run in disjoint PE row groups and overlap).
  - exp() on ScalarE (PSUM -> SBUF), no max-subtraction (|scores| <~ 50, safe
    in fp32).
  - AV: lhsT is v augmented with a ones column, so PSUM accumulates x^T
    unnormalized (rows 0-63 / 64-127) and the softmax denominator (row 64 for
    even heads, row 32 for odd heads) in the same accumulation group.
  - normalization: fast reciprocal of the denominator row, broadcast across
    partitions with a K=1 matmul, fused into the PSUM->SBUF evacuation.
  - projection: x^T chunks are the matmul lhsT directly; [N,256]@[256,1024]
    partial product is written out unreduced.
"""

import os
from contextlib import ExitStack

import ml_dtypes
import numpy as np

import concourse.bass as bass
import concourse.tile as tile
from concourse import bacc, mybir
from concourse._compat import with_exitstack
from concourse import bass_utils

F32 = mybir.dt.float32

B, N, C, H, D = 2, 2048, 1024, 16, 64
NCORES = 8
HPC = 4  # heads per core
NPAIR = HPC // 2


def _mm_dtypes():
    """PE dtypes for the three matmul groups.

    qk/proj: "f32" (exact, 4 cyc/col) or "f32r" (reduced precision, 1
    cyc/col).  av: additionally "bf16" — bf16 AV matmuls keep the PE's HAM
    clock gate warm (fp32r streaming does not count as PE activity), which
    doubles the effective PE clock for the whole kernel."""
    qk = os.environ.get("ATTN_KERNEL_QK_DT", "f32r")
    av = os.environ.get("ATTN_KERNEL_AV_DT", "bf16")
    pj = os.environ.get("ATTN_KERNEL_PJ_DT", "f16")
    m = {
        "f32": F32,
        "f32r": mybir.dt.float32r,
        "bf16": mybir.dt.bfloat16,
        "f16": mybir.dt.float16,
    }
    return m[qk], m[av], m[pj]


def _bcast_row(row_ap, nparts):
    """DRAM AP view replicating a 1D row across `nparts` partitions."""
    return bass.AP(
        tensor=row_ap.tensor,
        offset=row_ap.offset,
        ap=[[0, nparts], *row_ap.ap],
    )


@with_exitstack
def attention_body(ctx: ExitStack, tc: tile.TileContext, out, qt, kt, vp, wt):
    """Emit the per-core attention+projection program.

    APs (all fp32):
      out  [N, OW]          partial projection output
      qt   [NPAIR, 128, N]  q transposed, head pair stacked on partitions
      kt   [NPAIR, 128, N]  k transposed, same packing
      vp   [2*NPAIR, 128, NJ, 128]  v chunks as AV lhsT: for even heads v in
           cols 0:64 and ones in col 64; for odd heads v in cols 64:128 and
           ones in col 32 (so x^T lands on the partitions matching qt packing)
      wt   [NPAIR, 128, OW] proj_w slice, transposed to [channel, out]
    """
    nc = tc.nc
    P = 128
    pilot = int(os.environ.get("ATTN_KERNEL_PILOT", "0"))
    npair, _, n = qt.shape
    NJ = n // P          # key chunks
    HW = n // 2          # query half processed per inner loop
    NT = max(1, HW // 512)
    MS = HW // NT        # matmul free-dim chunk (<=512)
    OW = wt.shape[2]
    OT = max(1, OW // 512)
    OS = OW // OT

    sing = ctx.enter_context(tc.tile_pool(name="sing", bufs=1))
    probs_pool = ctx.enter_context(tc.tile_pool(name="probs", bufs=4))
    work = ctx.enter_context(tc.tile_pool(name="work", bufs=2))
    ost = ctx.enter_context(tc.tile_pool(name="ost", bufs=3))
    psum = ctx.enter_context(tc.tile_pool(name="psum", bufs=2, space="PSUM"))
    dram = ctx.enter_context(tc.tile_pool(name="dram", bufs=2, space="DRAM"))

    # HAM warm-up: dense plain-fp32 matmuls on a constant tile run during the
    # input DMA window (no data dependency) and lift the PE clock gate to
    # 2.4 GHz before the real f32r/bf16 stream begins
    nwarm = int(os.environ.get("ATTN_KERNEL_WARMUP", "6"))
    nburst = int(os.environ.get("ATTN_KERNEL_REWARM", "2"))
    wtile = None
    if nwarm or nburst:
        wtile = sing.tile([P, 512], F32, tag="warm", name="warm")
        nc.vector.memset(wtile, 1.0)

    def warm_burst(count, name):
        # plain-fp32 dummy matmuls re-lift the HAM clock gate; warmth then
        # persists ~50us into the f32r/bf16 stream (measured)
        pw = psum.tile([P, 512], F32, tag="ps", name=name)
        for w in range(count):
            nc.tensor.matmul(
                pw, lhsT=wtile[:, 0:128], rhs=wtile, start=True, stop=True
            )

    if nwarm:
        warm_burst(nwarm, "warmps")

    qts, kts, wts, xts, vps = [], [], [], [], []
    for p in range(npair):
        t = sing.tile([P, n], qt.dtype, tag=f"qt{p}", name=f"qts{p}")
        nc.sync.dma_start(t, qt[p])
        qts.append(t)
        t = sing.tile([P, n], kt.dtype, tag=f"kt{p}", name=f"kts{p}")
        nc.sync.dma_start(t, kt[p])
        kts.append(t)
        t = sing.tile([P, OW], wt.dtype, tag=f"wt{p}", name=f"wts{p}")
        nc.sync.dma_start(t, wt[p])
        wts.append(t)
        xts.append(sing.tile([P, n], wt.dtype, tag=f"xt{p}", name=f"xts{p}"))
    for h in range(2 * npair):
        t = sing.tile([P, NJ, P], vp.dtype, tag=f"vp{h}", name=f"vps{h}")
        nc.sync.dma_start(t, vp[h])
        vps.append(t)

    for Hi in range(2):
        h0 = Hi * HW
        for p in range(npair):
            po = [
                psum.tile([P, HW], F32, tag="po", name=f"po{Hi}{p}{a}")
                for a in range(2)
            ]
            def emit_qk(j):
                # QK for both heads, emission interleaved by row group so the
                # PE can overlap the K=64 matmuls of disjoint row halves
                pss = [
                    psum.tile([P, HW], F32, tag="ps", name=f"ps{Hi}{p}{j}{a}")
                    for a in range(2)
                ]
                for t in range(NT):
                    for a in range(2):
                        rows = slice(a * 64, a * 64 + 64)
                        nc.tensor.matmul(
                            pss[a][:, t * MS : (t + 1) * MS],
                            lhsT=kts[p][rows, j * P : (j + 1) * P],
                            rhs=qts[p][rows, h0 + t * MS : h0 + (t + 1) * MS],
                            start=True,
                            stop=True,
                        )
                return pss

            # software pipeline: emit QK for chunk j+1 before AV of chunk j so
            # the PE always has ready work behind the exp-gated AV matmuls
            pss = emit_qk(0)
            for j in range(NJ):
                pbs = []
                for a in range(2):
                    pb = probs_pool.tile(
                        [P, HW], vp.dtype, tag="pb", name=f"pb{Hi}{p}{j}{a}"
                    )
                    nc.scalar.activation(pb, pss[a], mybir.ActivationFunctionType.Exp)
                    pbs.append(pb)
                if j + 1 < NJ:
                    pss = emit_qk(j + 1)
                for a in range(2):
                    for t in range(NT):
                        nc.tensor.matmul(
                            po[a][:, t * MS : (t + 1) * MS],
                            lhsT=vps[2 * p + a][:, j, :],
                            rhs=pbs[a][:, t * MS : (t + 1) * MS],
                            start=(j == 0),
                            stop=(j == NJ - 1),
                        )
            # Evacuate PSUM immediately (denoms on ScalarE, x^T on VectorE) so
            # the po accumulators free up for the next head pair; the
            # broadcast + reciprocal + normalize then run asynchronously.
            dn = work.tile([P, HW], F32, tag="rc", name=f"rc{Hi}{p}")
            nc.vector.tensor_copy(dn[64:65, :], po[0][64:65, :])
            nc.vector.tensor_copy(dn[32:33, :], po[1][32:33, :])
            xu = work.tile([P, HW], F32, tag="xu", name=f"xu{Hi}{p}")
            nc.vector.tensor_copy(xu[0:64, :], po[0][0:64, :])
            nc.vector.tensor_copy(xu[64:128, :], po[1][64:128, :])
            dsc = dram.tile([2, HW], F32, tag="dsc", name=f"dsc{Hi}{p}")
            nc.sync.dma_start(dsc[0:1, :], dn[64:65, :])
            nc.sync.dma_start(dsc[1:2, :], dn[32:33, :])
            rbd = work.tile([P, HW], F32, tag="rbd", name=f"rbd{Hi}{p}")
            nc.sync.dma_start(rbd[0:64, :], _bcast_row(dsc[0], 64))
            nc.sync.dma_start(rbd[64:128, :], _bcast_row(dsc[1], 64))
            rb = work.tile([P, HW], F32, tag="rb", name=f"rb{Hi}{p}")
            rscr = work.tile([P, HW], F32, tag="rscr", name=f"rscr{Hi}{p}")
            nc.vector.reciprocal_approx_accurate(rb, rbd, rscr)
            nc.vector.tensor_mul(xts[p][:, h0 : h0 + HW], xu, rb)
            if nburst and not (Hi == 1 and p == npair - 1):
                warm_burst(nburst, f"rw{Hi}{p}")
    # projection: emitted after all attention so attention work is always
    # available behind it in the PE queue
    for i in range(n // P):
        pp = psum.tile([P, OW], F32, tag="ps", name=f"pp{i}")
        for cc in range(npair):
            for t in range(OT):
                nc.tensor.matmul(
                    pp[:, t * OS : (t + 1) * OS],
                    lhsT=xts[cc][:, i * P : (i + 1) * P],
                    rhs=wts[cc][:, t * OS : (t + 1) * OS],
                    start=(cc == 0),
                    stop=(cc == npair - 1),
                )
        ot = ost.tile([P, OW], F32, tag="ot", name=f"ot{i}")
        if i % 2 == 0:
            nc.vector.tensor_copy(ot, pp)
        else:
            nc.scalar.copy(ot, pp)
        nc.sync.dma_start(out[i * P : (i + 1) * P, :], ot)



def build_module(n=N, ow=C, npair=NPAIR):
    qkd, avd, pjd = _mm_dtypes()
    nc = bacc.Bacc("TRN2", target_bir_lowering=False, debug=False, num_devices=NCORES)
    nj = n // 128
    qt = nc.dram_tensor("qt", [npair, 128, n], qkd, kind="ExternalInput")
    kt = nc.dram_tensor("kt", [npair, 128, n], qkd, kind="ExternalInput")
    vp = nc.dram_tensor("vp", [2 * npair, 128, nj, 128], avd, kind="ExternalInput")
    wt = nc.dram_tensor("wt", [npair, 128, ow], pjd, kind="ExternalInput")
    out = nc.dram_tensor("out", [n, ow], F32, kind="ExternalOutput")
    with tile.TileContext(nc) as tc:
        attention_body(tc, out.ap(), qt.ap(), kt.ap(), vp.ap(), wt.ap())
    nc.compile()
    return nc


def shard_inputs(q, k, v, proj_w):
    """Build the 8 per-core input maps from the full tensors."""
    q = np.asarray(q, dtype=np.float32)
    k = np.asarray(k, dtype=np.float32)
    v = np.asarray(v, dtype=np.float32)
    proj_w = np.asarray(proj_w, dtype=np.float32)
    b_, n_, c_ = q.shape
    h_ = k.shape[1]
    d_ = c_ // h_
    nj = n_ // 128
    # [B, H, D, N]
    _np_dt = {"f32": np.float32, "f32r": np.float32, "bf16": ml_dtypes.bfloat16,
              "f16": np.float16}
    qk_np = _np_dt[os.environ.get("ATTN_KERNEL_QK_DT", "f32r")]
    qh = np.ascontiguousarray(
        q.reshape(b_, n_, h_, d_).transpose(0, 2, 3, 1).astype(qk_np)
    )
    kh = np.ascontiguousarray(k.transpose(0, 1, 3, 2).astype(qk_np))
    in_maps = []
    for c in range(NCORES):
        b = c // 4
        hh0 = HPC * (c % 4)
        qt = np.ascontiguousarray(qh[b, hh0 : hh0 + HPC].reshape(NPAIR, 128, n_))
        kt = np.ascontiguousarray(kh[b, hh0 : hh0 + HPC].reshape(NPAIR, 128, n_))
        avd = os.environ.get("ATTN_KERNEL_AV_DT", "bf16")
        vp_np = ml_dtypes.bfloat16 if avd == "bf16" else np.float32
        vp = np.zeros((HPC, 128, nj, 128), vp_np)
        for hh in range(HPC):
            vv = v[b, hh0 + hh].reshape(nj, 128, d_).transpose(1, 0, 2)
            if hh % 2 == 0:
                vp[hh][:, :, 0:64] = vv
                vp[hh][:, :, 64] = 1.0
            else:
                vp[hh][:, :, 64:128] = vv
                vp[hh][:, :, 32] = 1.0
        ch0 = hh0 * d_
        pj_np = _np_dt[os.environ.get("ATTN_KERNEL_PJ_DT", "f16")]
        wt = np.ascontiguousarray(
            proj_w[:, ch0 : ch0 + HPC * d_].T.reshape(NPAIR, 128, c_).astype(pj_np)
        )
        in_maps.append({"qt": qt, "kt": kt, "vp": vp, "wt": wt})
    return in_maps


def reduce_outputs(results, proj_b):
    """Sum the per-core partial projections per batch and add the bias."""
    outs = [np.asarray(r["out"], dtype=np.float32) for r in results]
    full = np.stack(
        [outs[0] + outs[1] + outs[2] + outs[3], outs[4] + outs[5] + outs[6] + outs[7]]
    )
    return (full + np.asarray(proj_b, dtype=np.float32)[None, None, :]).astype(
        np.float32
    )


_NC_CACHE = {}


def _get_module():
    if "nc" not in _NC_CACHE:
        _NC_CACHE["nc"] = build_module()
    return _NC_CACHE["nc"]


def kernel(q, k, v, proj_w, proj_b):
    nc = _get_module()
    in_maps = shard_inputs(q, k, v, proj_w)
    trace = bool(int(os.environ.get("ATTN_KERNEL_TRACE", "0")))
    kwargs = {}
    tmpdir = os.environ.get("ATTN_KERNEL_TMPDIR")
    if trace and tmpdir:
        os.makedirs(tmpdir, exist_ok=True)
        kwargs["tmpdir"] = tmpdir
    res = bass_utils.run_bass_kernel_spmd(
        nc, in_maps, core_ids=list(range(NCORES)), trace=trace, **kwargs
    )
    if trace:
        _NC_CACHE["last_results"] = res
    return reduce_outputs(res.results, proj_b)


# revision 29
# speedup vs baseline: 1.4212x; 1.0016x over previous
"""Multi-head self-attention block (B=2, N=2048, C=1024, H=16, D=64) + output
projection, sharded over 8 Trainium2 NeuronCores.

Sharding: core c handles batch b=c//4 and heads 4*(c%4)..4*(c%4)+3 (data +
head parallel).  The output projection is row-sharded over the input-channel
dim (each core multiplies its 256 attention channels into a full [N, 1024]
partial product); the 4 partials per batch are summed on the host (the
"all-reduce") and the bias is added there.

Device kernel layout (per core, fp32 throughout):
  - q, k are fed pre-transposed per head-pair: [128, N] tiles whose partition
    dim stacks the two heads' 64 attention dims.
  - scores_T[k_row, n] for a 128-row key chunk come from one K=64 matmul per
    head (the two heads run in disjoint PE row groups and overlap).
  - exp() on ScalarE (PSUM -> SBUF), no max-subtraction (|scores| <~ 50, safe
    in fp32).
  - AV: lhsT is v augmented with a ones column, so PSUM accumulates x^T
    unnormalized (rows 0-63 / 64-127) and the softmax denominator (row 64 for
    even heads, row 32 for odd heads) in the same accumulation group.
  - normalization: fast reciprocal of the denominator row, broadcast across
    partitions with a K=1 matmul, fused into the PSUM->SBUF evacuation.
  - projection: x^T chunks are the matmul lhsT directly; [N,256]@[256,1024]
    partial product is written out unreduced.
"""

import os
from contextlib import ExitStack

import ml_dtypes
import numpy as np

import concourse.bass as bass
import concourse.tile as tile
from concourse import bacc, mybir
from concourse._compat import with_exitstack
from concourse import bass_utils

F32 = mybir.dt.float32

B, N, C, H, D = 2, 2048, 1024, 16, 64
NCORES = 8
HPC = 4  # heads per core
NPAIR = HPC // 2


def _mm_dtypes():
    """PE dtypes for the three matmul groups.

    qk/proj: "f32" (exact, 4 cyc/col) or "f32r" (reduced precision, 1
    cyc/col).  av: additionally "bf16" — bf16 AV matmuls keep the PE's HAM
    clock gate warm (fp32r streaming does not count as PE activity), which
    doubles the effective PE clock for the whole kernel."""
    qk = os.environ.get("ATTN_KERNEL_QK_DT", "f32r")
    av = os.environ.get("ATTN_KERNEL_AV_DT", "bf16")
    pj = os.environ.get("ATTN_KERNEL_PJ_DT", "f16")
    m = {
        "f32": F32,
        "f32r": mybir.dt.float32r,
        "bf16": mybir.dt.bfloat16,
        "f16": mybir.dt.float16,
    }
    return m[qk], m[av], m[pj]


def _bcast_row(row_ap, nparts):
    """DRAM AP view replicating a 1D row across `nparts` partitions."""
    return bass.AP(
        tensor=row_ap.tensor,
        offset=row_ap.offset,
        ap=[[0, nparts], *row_ap.ap],
    )


@with_exitstack
def attention_body(ctx: ExitStack, tc: tile.TileContext, out, qt, kt, vp, wt):
    """Emit the per-core attention+projection program.

    APs (all fp32):
      out  [N, OW]          partial projection output
      qt   [NPAIR, 128, N]  q transposed, head pair stacked on partitions
      kt   [NPAIR, 128, N]  k transposed, same packing
      vp   [2*NPAIR, 128, NJ, 128]  v chunks as AV lhsT: for even heads v in
           cols 0:64 and ones in col 64; for odd heads v in cols 64:128 and
           ones in col 32 (so x^T lands on the partitions matching qt packing)
      wt   [NPAIR, 128, OW] proj_w slice, transposed to [channel, out]
    """
    nc = tc.nc
    P = 128
    pilot = int(os.environ.get("ATTN_KERNEL_PILOT", "0"))
    npair, _, n = qt.shape
    NJ = n // P          # key chunks
    HW = n // 2          # query half processed per inner loop
    NT = max(1, HW // 512)
    MS = HW // NT        # matmul free-dim chunk (<=512)
    OW = wt.shape[2]
    OT = max(1, OW // 512)
    OS = OW // OT

    sing = ctx.enter_context(tc.tile_pool(name="sing", bufs=1))
    probs_pool = ctx.enter_context(tc.tile_pool(name="probs", bufs=4))
    work = ctx.enter_context(tc.tile_pool(name="work", bufs=2))
    ost = ctx.enter_context(tc.tile_pool(name="ost", bufs=3))
    psum = ctx.enter_context(tc.tile_pool(name="psum", bufs=2, space="PSUM"))
    dram = ctx.enter_context(tc.tile_pool(name="dram", bufs=2, space="DRAM"))

    # HAM warm-up: dense plain-fp32 matmuls on a constant tile run during the
    # input DMA window (no data dependency) and lift the PE clock gate to
    # 2.4 GHz before the real f32r/bf16 stream begins
    nwarm = int(os.environ.get("ATTN_KERNEL_WARMUP", "6"))
    nburst = int(os.environ.get("ATTN_KERNEL_REWARM", "2"))
    wtile = None
    if nwarm or nburst:
        wtile = sing.tile([P, 512], F32, tag="warm", name="warm")
        nc.vector.memset(wtile, 1.0)

    def warm_burst(count, name):
        # plain-fp32 dummy matmuls re-lift the HAM clock gate; warmth then
        # persists ~50us into the f32r/bf16 stream (measured)
        pw = psum.tile([P, 512], F32, tag="ps", name=name)
        for w in range(count):
            nc.tensor.matmul(
                pw, lhsT=wtile[:, 0:128], rhs=wtile, start=True, stop=True
            )

    if nwarm:
        warm_burst(nwarm, "warmps")

    qts, kts, wts, xts, vps = [], [], [], [], []
    for p in range(npair):
        t = sing.tile([P, n], qt.dtype, tag=f"qt{p}", name=f"qts{p}")
        nc.sync.dma_start(t, qt[p])
        qts.append(t)
        t = sing.tile([P, n], kt.dtype, tag=f"kt{p}", name=f"kts{p}")
        nc.sync.dma_start(t, kt[p])
        kts.append(t)
        t = sing.tile([P, OW], wt.dtype, tag=f"wt{p}", name=f"wts{p}")
        nc.sync.dma_start(t, wt[p])
        wts.append(t)
        xts.append(sing.tile([P, n], wt.dtype, tag=f"xt{p}", name=f"xts{p}"))
    for h in range(2 * npair):
        t = sing.tile([P, NJ, P], vp.dtype, tag=f"vp{h}", name=f"vps{h}")
        nc.sync.dma_start(t, vp[h])
        vps.append(t)

    for Hi in range(2):
        h0 = Hi * HW
        for p in range(npair):
            po = [
                psum.tile([P, HW], F32, tag="po", name=f"po{Hi}{p}{a}")
                for a in range(2)
            ]
            def emit_qk(j):
                # QK for both heads, emission interleaved by row group so the
                # PE can overlap the K=64 matmuls of disjoint row halves
                pss = [
                    psum.tile([P, HW], F32, tag="ps", name=f"ps{Hi}{p}{j}{a}")
                    for a in range(2)
                ]
                for t in range(NT):
                    for a in range(2):
                        rows = slice(a * 64, a * 64 + 64)
                        nc.tensor.matmul(
                            pss[a][:, t * MS : (t + 1) * MS],
                            lhsT=kts[p][rows, j * P : (j + 1) * P],
                            rhs=qts[p][rows, h0 + t * MS : h0 + (t + 1) * MS],
                            start=True,
                            stop=True,
                        )
                return pss

            # software pipeline: emit QK for chunk j+1 before AV of chunk j so
            # the PE always has ready work behind the exp-gated AV matmuls
            pss = emit_qk(0)
            for j in range(NJ):
                pbs = []
                for a in range(2):
                    pb = probs_pool.tile(
                        [P, HW], vp.dtype, tag="pb", name=f"pb{Hi}{p}{j}{a}"
                    )
                    nc.scalar.activation(pb, pss[a], mybir.ActivationFunctionType.Exp)
                    pbs.append(pb)
                if j + 1 < NJ:
                    pss = emit_qk(j + 1)
                for a in range(2):
                    for t in range(NT):
                        nc.tensor.matmul(
                            po[a][:, t * MS : (t + 1) * MS],
                            lhsT=vps[2 * p + a][:, j, :],
                            rhs=pbs[a][:, t * MS : (t + 1) * MS],
                            start=(j == 0),
                            stop=(j == NJ - 1),
                        )
            # Evacuate PSUM immediately (denoms on ScalarE, x^T on VectorE) so
            # the po accumulators free up for the next head pair; the
            # broadcast + reciprocal + normalize then run asynchronously.
            dn = work.tile([P, HW], F32, tag="rc", name=f"rc{Hi}{p}")
            nc.vector.tensor_copy(dn[64:65, :], po[0][64:65, :])
            nc.vector.tensor_copy(dn[32:33, :], po[1][32:33, :])
            xu = work.tile([P, HW], F32, tag="xu", name=f"xu{Hi}{p}")
            nc.vector.tensor_copy(xu[0:64, :], po[0][0:64, :])
            nc.vector.tensor_copy(xu[64:128, :], po[1][64:128, :])
            dsc = dram.tile([2, HW], F32, tag="dsc", name=f"dsc{Hi}{p}")
            nc.sync.dma_start(dsc[0:1, :], dn[64:65, :])
            nc.sync.dma_start(dsc[1:2, :], dn[32:33, :])
            rbd = work.tile([P, HW], F32, tag="rbd", name=f"rbd{Hi}{p}")
            nc.sync.dma_start(rbd[0:64, :], _bcast_row(dsc[0], 64))
            nc.sync.dma_start(rbd[64:128, :], _bcast_row(dsc[1], 64))
            rb = work.tile([P, HW], F32, tag="rb", name=f"rb{Hi}{p}")
            rscr = work.tile([P, HW], F32, tag="rscr", name=f"rscr{Hi}{p}")
            nc.vector.reciprocal_approx_accurate(rb, rbd, rscr)
            nc.vector.tensor_mul(xts[p][:, h0 : h0 + HW], xu, rb)
            if nburst:
                warm_burst(nburst, f"rw{Hi}{p}")
    # projection: emitted after all attention so attention work is always
    # available behind it in the PE queue
    for i in range(n // P):
        pp = psum.tile([P, OW], F32, tag="ps", name=f"pp{i}")
        for cc in range(npair):
            for t in range(OT):
                nc.tensor.matmul(
                    pp[:, t * OS : (t + 1) * OS],
                    lhsT=xts[cc][:, i * P : (i + 1) * P],
                    rhs=wts[cc][:, t * OS : (t + 1) * OS],
                    start=(cc == 0),
                    stop=(cc == npair - 1),
                )
        ot = ost.tile([P, OW], F32, tag="ot", name=f"ot{i}")
        if i % 2 == 0:
            nc.vector.tensor_copy(ot, pp)
        else:
            nc.scalar.copy(ot, pp)
        nc.sync.dma_start(out[i * P : (i + 1) * P, :], ot)



def build_module(n=N, ow=C, npair=NPAIR):
    qkd, avd, pjd = _mm_dtypes()
    nc = bacc.Bacc("TRN2", target_bir_lowering=False, debug=False, num_devices=NCORES)
    nj = n // 128
    qt = nc.dram_tensor("qt", [npair, 128, n], qkd, kind="ExternalInput")
    kt = nc.dram_tensor("kt", [npair, 128, n], qkd, kind="ExternalInput")
    vp = nc.dram_tensor("vp", [2 * npair, 128, nj, 128], avd, kind="ExternalInput")
    wt = nc.dram_tensor("wt", [npair, 128, ow], pjd, kind="ExternalInput")
    out = nc.dram_tensor("out", [n, ow], F32, kind="ExternalOutput")
    with tile.TileContext(nc) as tc:
        attention_body(tc, out.ap(), qt.ap(), kt.ap(), vp.ap(), wt.ap())
    nc.compile()
    return nc


def shard_inputs(q, k, v, proj_w):
    """Build the 8 per-core input maps from the full tensors."""
    q = np.asarray(q, dtype=np.float32)
    k = np.asarray(k, dtype=np.float32)
    v = np.asarray(v, dtype=np.float32)
    proj_w = np.asarray(proj_w, dtype=np.float32)
    b_, n_, c_ = q.shape
    h_ = k.shape[1]
    d_ = c_ // h_
    nj = n_ // 128
    # [B, H, D, N]
    _np_dt = {"f32": np.float32, "f32r": np.float32, "bf16": ml_dtypes.bfloat16,
              "f16": np.float16}
    qk_np = _np_dt[os.environ.get("ATTN_KERNEL_QK_DT", "f32r")]
    qh = np.ascontiguousarray(
        q.reshape(b_, n_, h_, d_).transpose(0, 2, 3, 1).astype(qk_np)
    )
    kh = np.ascontiguousarray(k.transpose(0, 1, 3, 2).astype(qk_np))
    in_maps = []
    for c in range(NCORES):
        b = c // 4
        hh0 = HPC * (c % 4)
        qt = np.ascontiguousarray(qh[b, hh0 : hh0 + HPC].reshape(NPAIR, 128, n_))
        kt = np.ascontiguousarray(kh[b, hh0 : hh0 + HPC].reshape(NPAIR, 128, n_))
        avd = os.environ.get("ATTN_KERNEL_AV_DT", "bf16")
        vp_np = ml_dtypes.bfloat16 if avd == "bf16" else np.float32
        vp = np.zeros((HPC, 128, nj, 128), vp_np)
        for hh in range(HPC):
            vv = v[b, hh0 + hh].reshape(nj, 128, d_).transpose(1, 0, 2)
            if hh % 2 == 0:
                vp[hh][:, :, 0:64] = vv
                vp[hh][:, :, 64] = 1.0
            else:
                vp[hh][:, :, 64:128] = vv
                vp[hh][:, :, 32] = 1.0
        ch0 = hh0 * d_
        pj_np = _np_dt[os.environ.get("ATTN_KERNEL_PJ_DT", "f16")]
        wt = np.ascontiguousarray(
            proj_w[:, ch0 : ch0 + HPC * d_].T.reshape(NPAIR, 128, c_).astype(pj_np)
        )
        in_maps.append({"qt": qt, "kt": kt, "vp": vp, "wt": wt})
    return in_maps


def reduce_outputs(results, proj_b):
    """Sum the per-core partial projections per batch and add the bias."""
    outs = [np.asarray(r["out"], dtype=np.float32) for r in results]
    full = np.stack(
        [outs[0] + outs[1] + outs[2] + outs[3], outs[4] + outs[5] + outs[6] + outs[7]]
    )
    return (full + np.asarray(proj_b, dtype=np.float32)[None, None, :]).astype(
        np.float32
    )


_NC_CACHE = {}


def _get_module():
    if "nc" not in _NC_CACHE:
        _NC_CACHE["nc"] = build_module()
    return _NC_CACHE["nc"]


def kernel(q, k, v, proj_w, proj_b):
    nc = _get_module()
    in_maps = shard_inputs(q, k, v, proj_w)
    trace = bool(int(os.environ.get("ATTN_KERNEL_TRACE", "0")))
    kwargs = {}
    tmpdir = os.environ.get("ATTN_KERNEL_TMPDIR")
    if trace and tmpdir:
        os.makedirs(tmpdir, exist_ok=True)
        kwargs["tmpdir"] = tmpdir
    res = bass_utils.run_bass_kernel_spmd(
        nc, in_maps, core_ids=list(range(NCORES)), trace=trace, **kwargs
    )
    if trace:
        _NC_CACHE["last_results"] = res
    return reduce_outputs(res.results, proj_b)
